# revision 1
# baseline (speedup 1.0000x reference)
"""Trainium2 Bass kernel for a 2-layer GCN (AblationGCN) on 8 NeuronCores.

Contract: kernel(**inputs) takes the FULL unsharded inputs of the reference
(x [100000,165] f32, edge_index [2,1600000] int, W1,b1,W2,b2,Wc,bc) and
returns the FULL output [100000, 2] f32.

Strategy (SPMD, one Bass program on cores 0-7):
  - dst-range sharding: core c owns output nodes [c*12500, (c+1)*12500).
  - L1 dense (replicated): h1 = x @ W1 for all nodes via TensorE with
    host-transposed x; g1 = dinv*h1 stored as a bf16 node-major DRAM table.
  - L1 aggregation: edges (+self loops) grouped by dst block (128 nodes),
    each block padded to a uniform TB tiles of 128 edges. Per tile:
    indirect-DMA gather of g1[src] rows, a one-hot selection matrix S
    (iota == dstlocal on VectorE), and a TensorE matmul S^T @ G
    accumulating the block in PSUM. Epilogue: *dinv, +bias, relu.
  - a1 blocks are PE-transposed and exchanged with one AllGather.
  - L2 dense from the allgathered feature-major a1T; L2 aggregation same
    as L1; classifier a2T.T @ Wc + bc per block.
Host preprocessing computes degrees/dinv and the per-core uniform tile
metadata (indices, local dst one-hot keys).
"""
import numpy as np

P = 128
N = 100000
N_CORES = 8
IN_F = 165
HID = 128
SHARD = N // N_CORES            # 12500
NB = (SHARD + P - 1) // P       # 98
NPAD = ((N + P - 1) // P) * P   # 100096
SHARD_PAD = NB * P              # 12544
NPAD2 = N_CORES * SHARD_PAD     # 100352
NT1 = NPAD // P                 # 782


def _split_excess_waits(nc, max_waits=1):
    """This walrus build only accepts one sync-wait command per instruction;
    hoist extras onto NoOps placed just before the carrying instruction."""
    import concourse.mybir as mybir
    for f in nc.m.functions:
        for b in f.blocks:
            insts = b.instructions
            new_list = []
            changed = False
            for ins in insts:
                si = ins.sync_info
                if si is not None and len(si.on_wait) > max_waits:
                    waits = list(si.on_wait)
                    keep = waits[:max_waits]
                    extra = waits[max_waits:]
                    for ci in range(0, len(extra), max_waits):
                        chunk = extra[ci:ci + max_waits]
                        nop = mybir.InstNoOp(name=f"{ins.name}_wsplit{ci}",
                                             ins=[], outs=[])
                        nop.engine = ins.engine
                        nop.sync_info = mybir.SyncInfo(on_wait=chunk, on_update=[])
                        new_list.append(nop)
                    si.on_wait = keep
                    changed = True
                new_list.append(ins)
            if changed:
                b.instructions = new_list


def _build(tb, sbatch=16, xslab_tiles=64):
    import concourse.bass as bass
    import concourse.mybir as mybir
    import concourse.tile as tile
    F32 = mybir.dt.float32
    BF16 = mybir.dt.bfloat16
    I32 = mybir.dt.int32
    AF = mybir.ActivationFunctionType
    NT = NB * tb
    nc = bass.Bass()

    xt_a = nc.declare_dram_parameter("xt_a", [P, NPAD], BF16, isOutput=False)
    xt_b = nc.declare_dram_parameter("xt_b", [IN_F - P, NPAD], BF16, isOutput=False)
    w1a = nc.declare_dram_parameter("w1a", [P, HID], BF16, isOutput=False)
    w1b = nc.declare_dram_parameter("w1b", [IN_F - P, HID], BF16, isOutput=False)
    w2 = nc.declare_dram_parameter("w2", [HID, HID], BF16, isOutput=False)
    wc = nc.declare_dram_parameter("wc", [HID, 2], BF16, isOutput=False)
    b1rep = nc.declare_dram_parameter("b1rep", [P, HID], F32, isOutput=False)
    b2rep = nc.declare_dram_parameter("b2rep", [P, HID], F32, isOutput=False)
    bcrep = nc.declare_dram_parameter("bcrep", [P, 2], F32, isOutput=False)
    iota_rep = nc.declare_dram_parameter("iota_rep", [P, P], BF16, isOutput=False)
    ident = nc.declare_dram_parameter("ident", [P, P], BF16, isOutput=False)
    dinv_nt = nc.declare_dram_parameter("dinv_nt", [P, NT1], F32, isOutput=False)
    dinv_l2 = nc.declare_dram_parameter("dinv_l2", [P, N_CORES * NB], F32, isOutput=False)
    dinv_blk = nc.declare_dram_parameter("dinv_blk", [P, NB], F32, isOutput=False)
    dstloc = nc.declare_dram_parameter("dstloc", [P, NT], BF16, isOutput=False)
    idx1 = nc.declare_dram_parameter("idx1", [P, NT], I32, isOutput=False)
    idx2 = nc.declare_dram_parameter("idx2", [P, NT], I32, isOutput=False)
    out = nc.declare_dram_parameter("out", [SHARD, 2], F32, isOutput=True)

    with tile.TileContext(nc) as tc:
        with tc.tile_pool(name="const", bufs=1) as constp, \
             tc.tile_pool(name="dram", bufs=1, space="DRAM") as dramp:

            def load_const(name, param, shape, dt):
                t = constp.tile(shape, dt, name=name)
                nc.sync.dma_start(out=t[:], in_=param[:])
                return t

            w1a_s = load_const("w1a_s", w1a, [P, HID], BF16)
            w1b_s = load_const("w1b_s", w1b, [IN_F - P, HID], BF16)
            w2_s = load_const("w2_s", w2, [HID, HID], BF16)
            wc_s = load_const("wc_s", wc, [HID, 2], BF16)
            b1_s = load_const("b1_s", b1rep, [P, HID], F32)
            b2_s = load_const("b2_s", b2rep, [P, HID], F32)
            bc_s = load_const("bc_s", bcrep, [P, 2], F32)
            iota_s = load_const("iota_s", iota_rep, [P, P], BF16)
            id_s = load_const("id_s", ident, [P, P], BF16)
            dinv_nt_s = load_const("dinv_nt_s", dinv_nt, [P, NT1], F32)
            dinv_l2_s = load_const("dinv_l2_s", dinv_l2, [P, N_CORES * NB], F32)
            dinv_blk_s = load_const("dinv_blk_s", dinv_blk, [P, NB], F32)
            dstloc_s = load_const("dstloc_s", dstloc, [P, NT], BF16)
            idx1_s = load_const("idx1_s", idx1, [P, NT], I32)
            idx2_s = load_const("idx2_s", idx2, [P, NT], I32)

            g1_t = dramp.tile([NPAD, HID], BF16, name="g1_t")
            g2_t = dramp.tile([NPAD2, HID], BF16, name="g2_t")
            a1T_t = dramp.tile([P, SHARD_PAD], BF16, name="a1T_t")
            ag_t = dramp.tile([N_CORES * P, SHARD_PAD], BF16,
                              addr_space="Shared", name="ag_t")

            # ---------------- L1 dense ----------------
            with tc.tile_pool(name="xslab", bufs=2) as xsp, \
                 tc.tile_pool(name="gstage", bufs=2) as gsp, \
                 tc.tile_pool(name="psD", bufs=4, space="PSUM") as psD:
                t0 = 0
                while t0 < NT1:
                    tn = min(xslab_tiles, NT1 - t0)
                    xa = xsp.tile([P, xslab_tiles * P], BF16, tag="xa", name="xa")
                    nc.sync.dma_start(out=xa[:, :tn * P],
                                      in_=xt_a[:, t0 * P:(t0 + tn) * P])
                    xb = xsp.tile([IN_F - P, xslab_tiles * P], BF16, tag="xb",
                                  name="xb")
                    nc.sync.dma_start(out=xb[:, :tn * P],
                                      in_=xt_b[:, t0 * P:(t0 + tn) * P])
                    gst = gsp.tile([P, xslab_tiles * HID], BF16, tag="gst",
                                   name="gst")
                    for i in range(tn):
                        ps = psD.tile([P, HID], F32, tag="psD", name="psd")
                        nc.tensor.matmul(ps[:], lhsT=xa[:, i * P:(i + 1) * P],
                                         rhs=w1a_s[:], start=True, stop=False)
                        nc.tensor.matmul(ps[:], lhsT=xb[:, i * P:(i + 1) * P],
                                         rhs=w1b_s[:], start=False, stop=True)
                        nc.scalar.activation(gst[:, i * HID:(i + 1) * HID], ps[:],
                                             AF.Copy,
                                             scale=dinv_nt_s[:, t0 + i:t0 + i + 1])
                    nc.sync.dma_start(
                        out=g1_t[t0 * P:(t0 + tn) * P, :].rearrange(
                            "(t p) f -> p t f", p=P),
                        in_=gst[:, :tn * HID].rearrange("p (t f) -> p t f", f=HID))
                    t0 += tn

            def aggregation(g_tab, idx_s, bias_s, out_blk_cb, phase):
                with tc.tile_pool(name=f"gt{phase}", bufs=8) as gtp, \
                     tc.tile_pool(name=f"sp{phase}", bufs=3) as spp, \
                     tc.tile_pool(name=f"ps{phase}", bufs=4, space="PSUM") as psp, \
                     tc.tile_pool(name=f"ep{phase}", bufs=3) as epp:
                    S = None
                    for b in range(NB):
                        ps = psp.tile([P, HID], F32, tag="ps", name="ps")
                        for t in range(tb):
                            gi = b * tb + t
                            if gi % sbatch == 0:
                                nw = min(sbatch, NT - gi)
                                S = spp.tile([P, sbatch * P], BF16, tag="S",
                                             name="S")
                                iota_b = bass.AP(
                                    iota_s[:].tensor, iota_s[:].offset,
                                    [iota_s[:].ap[0], [0, nw], iota_s[:].ap[1]])
                                dst_b = dstloc_s[:, gi:gi + nw].to_broadcast(
                                    [P, nw, P])
                                nc.vector.tensor_tensor(
                                    out=S[:, :nw * P].rearrange(
                                        "p (t f) -> p t f", f=P),
                                    in0=iota_b, in1=dst_b,
                                    op=mybir.AluOpType.is_equal)
                            G = gtp.tile([P, HID], BF16, tag="G", name="G")
                            nc.gpsimd.indirect_dma_start(
                                out=G[:], out_offset=None, in_=g_tab[:],
                                in_offset=bass.IndirectOffsetOnAxis(
                                    ap=idx_s[:, gi:gi + 1], axis=0))
                            nc.tensor.matmul(
                                ps[:],
                                lhsT=S[:, (gi % sbatch) * P:(gi % sbatch + 1) * P],
                                rhs=G[:], start=(t == 0), stop=(t == tb - 1))
                        t1 = epp.tile([P, HID], F32, tag="t1", name="t1")
                        nc.scalar.activation(t1[:], ps[:], AF.Copy,
                                             scale=dinv_blk_s[:, b:b + 1])
                        t2 = epp.tile([P, HID], F32, tag="t2", name="t2")
                        nc.vector.tensor_tensor(out=t2[:], in0=t1[:], in1=bias_s[:],
                                                op=mybir.AluOpType.add)
                        a_sb = epp.tile([P, HID], BF16, tag="a_sb", name="a_sb")
                        nc.vector.tensor_scalar_max(out=a_sb[:], in0=t2[:],
                                                    scalar1=0.0)
                        out_blk_cb(b, a_sb)

            # ---------------- L1 agg -> a1T -> AllGather ----------------
            with tc.tile_pool(name="a1Ts", bufs=1) as a1sp, \
                 tc.tile_pool(name="psT", bufs=2, space="PSUM") as psT:
                a1T_stage = a1sp.tile([P, SHARD_PAD], BF16, name="a1T_stage")

                def l1_out(b, a_sb):
                    pT = psT.tile([P, P], BF16, tag="pT", name="pT")
                    nc.tensor.transpose(pT[:], a_sb[:], id_s[:])
                    nc.vector.tensor_copy(out=a1T_stage[:, b * P:(b + 1) * P],
                                          in_=pT[:])

                aggregation(g1_t, idx1_s, b1_s, l1_out, "A")
                nc.sync.dma_start(out=a1T_t[:], in_=a1T_stage[:])

            nc.gpsimd.collective_compute(
                "AllGather", mybir.AluOpType.bypass,
                replica_groups=[list(range(N_CORES))],
                ins=[a1T_t[:]], outs=[ag_t[:]])

            # ---------------- L2 dense ----------------
            with tc.tile_pool(name="l2slab", bufs=2) as l2sp, \
                 tc.tile_pool(name="gstage2", bufs=2) as gsp2, \
                 tc.tile_pool(name="psD2", bufs=4, space="PSUM") as psD2:
                for cb in range(N_CORES):
                    t0 = 0
                    while t0 < NB:
                        tn = min(32, NB - t0)
                        lh = l2sp.tile([P, 32 * P], BF16, tag="lh", name="lh")
                        nc.sync.dma_start(out=lh[:, :tn * P],
                                          in_=ag_t[cb * P:(cb + 1) * P,
                                                   t0 * P:(t0 + tn) * P])
                        gst = gsp2.tile([P, 32 * HID], BF16, tag="gst2",
                                        name="gst2")
                        for i in range(tn):
                            ps = psD2.tile([P, HID], F32, tag="psD2", name="psd2")
                            nc.tensor.matmul(ps[:], lhsT=lh[:, i * P:(i + 1) * P],
                                             rhs=w2_s[:], start=True, stop=True)
                            nc.scalar.activation(
                                gst[:, i * HID:(i + 1) * HID], ps[:], AF.Copy,
                                scale=dinv_l2_s[:, cb * NB + t0 + i:
                                                cb * NB + t0 + i + 1])
                        r0 = cb * SHARD_PAD + t0 * P
                        nc.sync.dma_start(
                            out=g2_t[r0:r0 + tn * P, :].rearrange(
                                "(t p) f -> p t f", p=P),
                            in_=gst[:, :tn * HID].rearrange("p (t f) -> p t f",
                                                            f=HID))
                        t0 += tn

            # ---------------- L2 agg + classifier ----------------
            with tc.tile_pool(name="psT2", bufs=2, space="PSUM") as psT2, \
                 tc.tile_pool(name="clsb", bufs=3) as clsp, \
                 tc.tile_pool(name="psC", bufs=2, space="PSUM") as psC:

                def l2_out(b, a_sb):
                    pT = psT2.tile([P, P], BF16, tag="pT2", name="pT2")
                    nc.tensor.transpose(pT[:], a_sb[:], id_s[:])
                    a2T = clsp.tile([P, P], BF16, tag="a2T", name="a2T")
                    nc.vector.tensor_copy(out=a2T[:], in_=pT[:])
                    pc = psC.tile([P, 2], F32, tag="pC", name="pC")
                    nc.tensor.matmul(pc[:], lhsT=a2T[:], rhs=wc_s[:],
                                     start=True, stop=True)
                    ob = clsp.tile([P, 2], F32, tag="ob", name="ob")
                    nc.vector.tensor_tensor(out=ob[:], in0=pc[:], in1=bc_s[:],
                                            op=mybir.AluOpType.add)
                    nrows = min(P, SHARD - b * P)
                    nc.sync.dma_start(out=out[b * P:b * P + nrows, :],
                                      in_=ob[:nrows, :])

                aggregation(g2_t, idx2_s, b2_s, l2_out, "B")

    mybir.codegen_inst_isa_subclasses(nc)
    _split_excess_waits(nc)
    return nc


def _prepare(x, edge_index, W1, b1, W2, b2, Wc, bc):
    import ml_dtypes
    bf = ml_dtypes.bfloat16
    x = np.asarray(x, np.float32)
    src = np.asarray(edge_index[0], dtype=np.int64)
    dst = np.asarray(edge_index[1], dtype=np.int64)
    deg = np.bincount(dst, minlength=N).astype(np.float32) + 1.0
    dinv = 1.0 / np.sqrt(deg)
    allsrc = np.concatenate([src, np.arange(N, dtype=np.int64)])
    alldst = np.concatenate([dst, np.arange(N, dtype=np.int64)])

    per_core = []
    tb = 0
    for cc in range(N_CORES):
        lo, hi = cc * SHARD, (cc + 1) * SHARD
        m = (alldst >= lo) & (alldst < hi)
        s = allsrc[m]
        d = alldst[m] - lo
        blk = d // P
        order = np.argsort(blk, kind="stable")
        s, d, blk = s[order], d[order], blk[order]
        counts = np.bincount(blk, minlength=NB)
        tb = max(tb, int(np.ceil(counts.max() / P)))
        per_core.append((s, d, counts))
    NT = NB * tb

    dinv_pad = np.ones(NPAD, np.float32)
    dinv_pad[:N] = dinv
    xt = np.zeros((IN_F, NPAD), np.float32)
    xt[:, :N] = x.T
    common = {
        "xt_a": xt[:P].astype(bf),
        "xt_b": xt[P:].astype(bf),
        "w1a": np.asarray(W1, np.float32)[:P].astype(bf),
        "w1b": np.asarray(W1, np.float32)[P:].astype(bf),
        "w2": np.asarray(W2, np.float32).astype(bf),
        "wc": np.asarray(Wc, np.float32).astype(bf),
        "b1rep": np.broadcast_to(np.asarray(b1, np.float32), (P, HID)).copy(),
        "b2rep": np.broadcast_to(np.asarray(b2, np.float32), (P, HID)).copy(),
        "bcrep": np.broadcast_to(np.asarray(bc, np.float32), (P, 2)).copy(),
        "iota_rep": np.broadcast_to(np.arange(P, dtype=np.float32),
                                    (P, P)).astype(bf),
        "ident": np.eye(P, dtype=np.float32).astype(bf),
        "dinv_nt": dinv_pad.reshape(NT1, P).T.copy(),
    }
    d2 = np.ones((N_CORES, SHARD_PAD), np.float32)
    for cc in range(N_CORES):
        d2[cc, :SHARD] = dinv[cc * SHARD:(cc + 1) * SHARD]
    common["dinv_l2"] = d2.reshape(N_CORES * NB, P).T.copy()

    in_maps = []
    for cc in range(N_CORES):
        s, d, counts = per_core[cc]
        idx_u = np.zeros((NT, P), np.int64)
        dst_u = np.full((NT, P), -1.0, np.float32)
        offs = np.concatenate([[0], np.cumsum(counts)])
        for b in range(NB):
            es = s[offs[b]:offs[b + 1]]
            ed = d[offs[b]:offs[b + 1]] % P
            nloc = len(es)
            buf_s = np.zeros(tb * P, np.int64)
            buf_d = np.full(tb * P, -1.0, np.float32)
            buf_s[:nloc] = es
            buf_d[:nloc] = ed
            idx_u[b * tb:(b + 1) * tb] = buf_s.reshape(tb, P)
            dst_u[b * tb:(b + 1) * tb] = buf_d.reshape(tb, P)
        idx_u = idx_u.T.copy()
        dst_u = dst_u.T.copy()
        idx2_u = (idx_u // SHARD) * SHARD_PAD + (idx_u % SHARD)
        lo = cc * SHARD
        dv = np.ones(SHARD_PAD, np.float32)
        dv[:SHARD] = dinv[lo:lo + SHARD]
        m = dict(common)
        m["idx1"] = idx_u.astype(np.int32)
        m["idx2"] = idx2_u.astype(np.int32)
        m["dstloc"] = dst_u.astype(bf)
        m["dinv_blk"] = dv.reshape(NB, P).T.copy()
        in_maps.append(m)
    return tb, in_maps


class _Runner:
    """Compile the Bass SPMD program once and execute it on cores 0-7 via
    the PJRT path (modeled on concourse.bass2jax.run_bass_via_pjrt)."""

    def __init__(self, nc, n_cores=8):
        import jax
        import concourse.mybir as mybir
        from jax.sharding import Mesh, PartitionSpec
        from jax.experimental.shard_map import shard_map
        from concourse.bass2jax import (_bass_exec_p, partition_id_tensor,
                                        install_neuronx_cc_hook)
        install_neuronx_cc_hook()
        self.jax = jax
        self.n_cores = n_cores
        in_names, out_names, out_avals = [], [], []
        partition_name = (nc.partition_id_tensor.name
                          if nc.partition_id_tensor else None)
        for alloc in nc.m.functions[0].allocations:
            if not isinstance(alloc, mybir.MemoryLocationSet):
                continue
            name = alloc.memorylocations[0].name
            if alloc.kind == "ExternalInput":
                if name != partition_name:
                    in_names.append(name)
            elif alloc.kind == "ExternalOutput":
                out_names.append(name)
                out_avals.append(jax.core.ShapedArray(
                    tuple(alloc.tensor_shape), mybir.dt.np(alloc.dtype)))
        self.in_names, self.out_names, self.out_avals = \
            in_names, out_names, out_avals
        n_params = len(in_names)
        all_in = list(in_names) + list(out_names)
        if partition_name is not None:
            all_in.append(partition_name)

        def _body(*args):
            operands = list(args)
            if partition_name is not None:
                operands.append(partition_id_tensor())
            outs = _bass_exec_p.bind(
                *operands, out_avals=tuple(out_avals), in_names=tuple(all_in),
                out_names=tuple(out_names), lowering_input_output_aliases=(),
                sim_require_finite=True, sim_require_nnan=True, nc=nc)
            return tuple(outs)

        devices = jax.devices()[:n_cores]
        self.mesh = Mesh(np.asarray(devices), ("core",))
        n_outs = len(out_avals)
        in_specs = (PartitionSpec("core"),) * (n_params + n_outs)
        out_specs = (PartitionSpec("core"),) * n_outs
        self.fn = jax.jit(
            shard_map(_body, mesh=self.mesh, in_specs=in_specs,
                      out_specs=out_specs, check_rep=False),
            keep_unused=True)

    def prep_inputs(self, in_maps):
        import jax
        from jax.sharding import NamedSharding, PartitionSpec
        concat = [np.concatenate([np.asarray(m[name]) for m in in_maps], axis=0)
                  for name in self.in_names]
        zeros = [np.zeros((self.n_cores * a.shape[0], *a.shape[1:]), a.dtype)
                 for a in self.out_avals]
        sharding = NamedSharding(self.mesh, PartitionSpec("core"))
        return [jax.device_put(a, sharding) for a in concat + zeros]

    def run(self, dev_args):
        outs = self.fn(*dev_args)
        self.jax.block_until_ready(outs)
        return outs

    def results(self, outs):
        res = []
        for c in range(self.n_cores):
            d = {}
            for i, name in enumerate(self.out_names):
                d[name] = np.asarray(outs[i]).reshape(
                    self.n_cores, *self.out_avals[i].shape)[c]
            res.append(d)
        return res


_CACHED = {}


def kernel(x, edge_index, W1, b1, W2, b2, Wc, bc):
    tb, in_maps = _prepare(x, edge_index, W1, b1, W2, b2, Wc, bc)
    key = tb
    if key not in _CACHED:
        nc = _build(tb)
        _CACHED[key] = _Runner(nc)
    r = _CACHED[key]
    dev = r.prep_inputs(in_maps)
    outs = r.run(dev)
    res = r.results(outs)
    full = np.concatenate([res[c]["out"] for c in range(N_CORES)], axis=0)
    return full.astype(np.float32)


# revision 2
# speedup vs baseline: 1.0897x; 1.0897x over previous
"""Trainium2 Bass kernel for a 2-layer GCN (AblationGCN) on 8 NeuronCores.

Contract: kernel(**inputs) takes the FULL unsharded inputs of the reference
(x [100000,165] f32, edge_index [2,1600000] int, W1,b1,W2,b2,Wc,bc) and
returns the FULL output [100000, 2] f32.

Strategy (SPMD, one Bass program on cores 0-7):
  - dst-range sharding: core c owns output nodes [c*12500, (c+1)*12500).
  - L1 dense (replicated): h1 = x @ W1 for all nodes via TensorE with
    host-transposed x; g1 = dinv*h1 stored as a bf16 node-major DRAM table.
  - L1 aggregation: edges (+self loops) grouped by dst block (128 nodes),
    each block padded to a uniform TB tiles of 128 edges. Per tile:
    indirect-DMA gather of g1[src] rows, a one-hot selection matrix S
    (iota == dstlocal on VectorE), and a TensorE matmul S^T @ G
    accumulating the block in PSUM. Epilogue: *dinv, +bias, relu.
  - a1 blocks are PE-transposed and exchanged with one AllGather.
  - L2 dense from the allgathered feature-major a1T; L2 aggregation same
    as L1; classifier a2T.T @ Wc + bc per block.
Host preprocessing computes degrees/dinv and the per-core uniform tile
metadata (indices, local dst one-hot keys).
"""
import numpy as np

P = 128
N = 100000
N_CORES = 8
IN_F = 165
HID = 128
SHARD = N // N_CORES            # 12500
NB = (SHARD + P - 1) // P       # 98
NPAD = ((N + P - 1) // P) * P   # 100096
SHARD_PAD = NB * P              # 12544
NPAD2 = N_CORES * SHARD_PAD     # 100352
NT1 = NPAD // P                 # 782


def _split_excess_waits(nc, max_waits=1):
    """This walrus build only accepts one sync-wait command per instruction;
    hoist extras onto NoOps placed just before the carrying instruction."""
    import concourse.mybir as mybir
    for f in nc.m.functions:
        for b in f.blocks:
            insts = b.instructions
            new_list = []
            changed = False
            for ins in insts:
                si = ins.sync_info
                if si is not None and len(si.on_wait) > max_waits:
                    waits = list(si.on_wait)
                    keep = waits[:max_waits]
                    extra = waits[max_waits:]
                    for ci in range(0, len(extra), max_waits):
                        chunk = extra[ci:ci + max_waits]
                        nop = mybir.InstNoOp(name=f"{ins.name}_wsplit{ci}",
                                             ins=[], outs=[])
                        nop.engine = ins.engine
                        nop.sync_info = mybir.SyncInfo(on_wait=chunk, on_update=[])
                        new_list.append(nop)
                    si.on_wait = keep
                    changed = True
                new_list.append(ins)
            if changed:
                b.instructions = new_list


def _build(tb, sbatch=16, xslab_tiles=64):
    import concourse.bass as bass
    import concourse.mybir as mybir
    import concourse.tile as tile
    F32 = mybir.dt.float32
    BF16 = mybir.dt.bfloat16
    I32 = mybir.dt.int32
    AF = mybir.ActivationFunctionType
    NT = NB * tb
    nc = bass.Bass()

    xt_a = nc.declare_dram_parameter("xt_a", [P, NPAD], BF16, isOutput=False)
    xt_b = nc.declare_dram_parameter("xt_b", [IN_F - P, NPAD], BF16, isOutput=False)
    w1a = nc.declare_dram_parameter("w1a", [P, HID], BF16, isOutput=False)
    w1b = nc.declare_dram_parameter("w1b", [IN_F - P, HID], BF16, isOutput=False)
    w2 = nc.declare_dram_parameter("w2", [HID, HID], BF16, isOutput=False)
    wc = nc.declare_dram_parameter("wc", [HID, 2], BF16, isOutput=False)
    b1rep = nc.declare_dram_parameter("b1rep", [P, HID], F32, isOutput=False)
    b2rep = nc.declare_dram_parameter("b2rep", [P, HID], F32, isOutput=False)
    bcrep = nc.declare_dram_parameter("bcrep", [P, 2], F32, isOutput=False)
    iota_rep = nc.declare_dram_parameter("iota_rep", [P, P], BF16, isOutput=False)
    ident = nc.declare_dram_parameter("ident", [P, P], BF16, isOutput=False)
    dinv_nt = nc.declare_dram_parameter("dinv_nt", [P, NT1], F32, isOutput=False)
    dinv_l2 = nc.declare_dram_parameter("dinv_l2", [P, N_CORES * NB], F32, isOutput=False)
    dinv_blk = nc.declare_dram_parameter("dinv_blk", [P, NB], F32, isOutput=False)
    dstloc = nc.declare_dram_parameter("dstloc", [P, NT], BF16, isOutput=False)
    idx1 = nc.declare_dram_parameter("idx1", [P, NT], I32, isOutput=False)
    idx2 = nc.declare_dram_parameter("idx2", [P, NT], I32, isOutput=False)
    out = nc.declare_dram_parameter("out", [SHARD, 2], F32, isOutput=True)

    with tile.TileContext(nc) as tc:
        with tc.tile_pool(name="const", bufs=1) as constp, \
             tc.tile_pool(name="dram", bufs=1, space="DRAM") as dramp:

            def load_const(name, param, shape, dt):
                t = constp.tile(shape, dt, name=name)
                nc.sync.dma_start(out=t[:], in_=param[:])
                return t

            w1a_s = load_const("w1a_s", w1a, [P, HID], BF16)
            w1b_s = load_const("w1b_s", w1b, [IN_F - P, HID], BF16)
            w2_s = load_const("w2_s", w2, [HID, HID], BF16)
            wc_s = load_const("wc_s", wc, [HID, 2], BF16)
            b1_s = load_const("b1_s", b1rep, [P, HID], F32)
            b2_s = load_const("b2_s", b2rep, [P, HID], F32)
            bc_s = load_const("bc_s", bcrep, [P, 2], F32)
            iota_s = load_const("iota_s", iota_rep, [P, P], BF16)
            id_s = load_const("id_s", ident, [P, P], BF16)
            dinv_nt_s = load_const("dinv_nt_s", dinv_nt, [P, NT1], F32)
            dinv_l2_s = load_const("dinv_l2_s", dinv_l2, [P, N_CORES * NB], F32)
            dinv_blk_s = load_const("dinv_blk_s", dinv_blk, [P, NB], F32)
            dstloc_s = load_const("dstloc_s", dstloc, [P, NT], BF16)
            idx1_s = load_const("idx1_s", idx1, [P, NT], I32)
            idx2_s = load_const("idx2_s", idx2, [P, NT], I32)

            g1_t = dramp.tile([NPAD, HID], BF16, name="g1_t")
            g2_t = dramp.tile([NPAD2, HID], BF16, name="g2_t")
            a1T_t = dramp.tile([P, SHARD_PAD], BF16, name="a1T_t")
            ag_t = dramp.tile([N_CORES * P, SHARD_PAD], BF16,
                              addr_space="Shared", name="ag_t")

            # ---------------- L1 dense ----------------
            with tc.tile_pool(name="xslab", bufs=2) as xsp, \
                 tc.tile_pool(name="gstage", bufs=2) as gsp, \
                 tc.tile_pool(name="psD", bufs=4, space="PSUM") as psD:
                t0 = 0
                while t0 < NT1:
                    tn = min(xslab_tiles, NT1 - t0)
                    xa = xsp.tile([P, xslab_tiles * P], BF16, tag="xa", name="xa")
                    nc.sync.dma_start(out=xa[:, :tn * P],
                                      in_=xt_a[:, t0 * P:(t0 + tn) * P])
                    xb = xsp.tile([IN_F - P, xslab_tiles * P], BF16, tag="xb",
                                  name="xb")
                    nc.sync.dma_start(out=xb[:, :tn * P],
                                      in_=xt_b[:, t0 * P:(t0 + tn) * P])
                    gst = gsp.tile([P, xslab_tiles * HID], BF16, tag="gst",
                                   name="gst")
                    for i in range(tn):
                        ps = psD.tile([P, HID], F32, tag="psD", name="psd")
                        nc.tensor.matmul(ps[:], lhsT=xa[:, i * P:(i + 1) * P],
                                         rhs=w1a_s[:], start=True, stop=False)
                        nc.tensor.matmul(ps[:], lhsT=xb[:, i * P:(i + 1) * P],
                                         rhs=w1b_s[:], start=False, stop=True)
                        nc.scalar.activation(gst[:, i * HID:(i + 1) * HID], ps[:],
                                             AF.Copy,
                                             scale=dinv_nt_s[:, t0 + i:t0 + i + 1])
                    nc.sync.dma_start(
                        out=g1_t[t0 * P:(t0 + tn) * P, :].rearrange(
                            "(t p) f -> p t f", p=P),
                        in_=gst[:, :tn * HID].rearrange("p (t f) -> p t f", f=HID))
                    t0 += tn

            def aggregation(g_tab, idx_s, bias_s, out_blk_cb, phase):
                with tc.tile_pool(name=f"gt{phase}", bufs=16) as gtp, \
                     tc.tile_pool(name=f"sp{phase}", bufs=3) as spp, \
                     tc.tile_pool(name=f"ps{phase}", bufs=4, space="PSUM") as psp, \
                     tc.tile_pool(name=f"ep{phase}", bufs=3) as epp:
                    S = None
                    for b in range(NB):
                        ps = psp.tile([P, HID], F32, tag="ps", name="ps")
                        for t in range(tb):
                            gi = b * tb + t
                            if gi % sbatch == 0:
                                nw = min(sbatch, NT - gi)
                                S = spp.tile([P, sbatch * P], BF16, tag="S",
                                             name="S")
                                iota_b = bass.AP(
                                    iota_s[:].tensor, iota_s[:].offset,
                                    [iota_s[:].ap[0], [0, nw], iota_s[:].ap[1]])
                                dst_b = dstloc_s[:, gi:gi + nw].to_broadcast(
                                    [P, nw, P])
                                nc.vector.tensor_tensor(
                                    out=S[:, :nw * P].rearrange(
                                        "p (t f) -> p t f", f=P),
                                    in0=iota_b, in1=dst_b,
                                    op=mybir.AluOpType.is_equal)
                            G = gtp.tile([P, HID], BF16, tag="G", name="G")
                            nc.gpsimd.indirect_dma_start(
                                out=G[:], out_offset=None, in_=g_tab[:],
                                in_offset=bass.IndirectOffsetOnAxis(
                                    ap=idx_s[:, gi:gi + 1], axis=0))
                            nc.tensor.matmul(
                                ps[:],
                                lhsT=S[:, (gi % sbatch) * P:(gi % sbatch + 1) * P],
                                rhs=G[:], start=(t == 0), stop=(t == tb - 1))
                        t1 = epp.tile([P, HID], F32, tag="t1", name="t1")
                        nc.scalar.activation(t1[:], ps[:], AF.Copy,
                                             scale=dinv_blk_s[:, b:b + 1])
                        t2 = epp.tile([P, HID], F32, tag="t2", name="t2")
                        nc.vector.tensor_tensor(out=t2[:], in0=t1[:], in1=bias_s[:],
                                                op=mybir.AluOpType.add)
                        a_sb = epp.tile([P, HID], BF16, tag="a_sb", name="a_sb")
                        nc.vector.tensor_scalar_max(out=a_sb[:], in0=t2[:],
                                                    scalar1=0.0)
                        out_blk_cb(b, a_sb)

            # ---------------- L1 agg -> a1T -> AllGather ----------------
            with tc.tile_pool(name="a1Ts", bufs=1) as a1sp, \
                 tc.tile_pool(name="psT", bufs=2, space="PSUM") as psT:
                a1T_stage = a1sp.tile([P, SHARD_PAD], BF16, name="a1T_stage")

                def l1_out(b, a_sb):
                    pT = psT.tile([P, P], BF16, tag="pT", name="pT")
                    nc.tensor.transpose(pT[:], a_sb[:], id_s[:])
                    nc.vector.tensor_copy(out=a1T_stage[:, b * P:(b + 1) * P],
                                          in_=pT[:])

                aggregation(g1_t, idx1_s, b1_s, l1_out, "A")
                nc.sync.dma_start(out=a1T_t[:], in_=a1T_stage[:])

            nc.gpsimd.collective_compute(
                "AllGather", mybir.AluOpType.bypass,
                replica_groups=[list(range(N_CORES))],
                ins=[a1T_t[:]], outs=[ag_t[:]])

            # ---------------- L2 dense ----------------
            with tc.tile_pool(name="l2slab", bufs=2) as l2sp, \
                 tc.tile_pool(name="gstage2", bufs=2) as gsp2, \
                 tc.tile_pool(name="psD2", bufs=4, space="PSUM") as psD2:
                for cb in range(N_CORES):
                    t0 = 0
                    while t0 < NB:
                        tn = min(32, NB - t0)
                        lh = l2sp.tile([P, 32 * P], BF16, tag="lh", name="lh")
                        nc.sync.dma_start(out=lh[:, :tn * P],
                                          in_=ag_t[cb * P:(cb + 1) * P,
                                                   t0 * P:(t0 + tn) * P])
                        gst = gsp2.tile([P, 32 * HID], BF16, tag="gst2",
                                        name="gst2")
                        for i in range(tn):
                            ps = psD2.tile([P, HID], F32, tag="psD2", name="psd2")
                            nc.tensor.matmul(ps[:], lhsT=lh[:, i * P:(i + 1) * P],
                                             rhs=w2_s[:], start=True, stop=True)
                            nc.scalar.activation(
                                gst[:, i * HID:(i + 1) * HID], ps[:], AF.Copy,
                                scale=dinv_l2_s[:, cb * NB + t0 + i:
                                                cb * NB + t0 + i + 1])
                        r0 = cb * SHARD_PAD + t0 * P
                        nc.sync.dma_start(
                            out=g2_t[r0:r0 + tn * P, :].rearrange(
                                "(t p) f -> p t f", p=P),
                            in_=gst[:, :tn * HID].rearrange("p (t f) -> p t f",
                                                            f=HID))
                        t0 += tn

            # ---------------- L2 agg + classifier ----------------
            with tc.tile_pool(name="psT2", bufs=2, space="PSUM") as psT2, \
                 tc.tile_pool(name="clsb", bufs=3) as clsp, \
                 tc.tile_pool(name="psC", bufs=2, space="PSUM") as psC:

                def l2_out(b, a_sb):
                    pT = psT2.tile([P, P], BF16, tag="pT2", name="pT2")
                    nc.tensor.transpose(pT[:], a_sb[:], id_s[:])
                    a2T = clsp.tile([P, P], BF16, tag="a2T", name="a2T")
                    nc.vector.tensor_copy(out=a2T[:], in_=pT[:])
                    pc = psC.tile([P, 2], F32, tag="pC", name="pC")
                    nc.tensor.matmul(pc[:], lhsT=a2T[:], rhs=wc_s[:],
                                     start=True, stop=True)
                    ob = clsp.tile([P, 2], F32, tag="ob", name="ob")
                    nc.vector.tensor_tensor(out=ob[:], in0=pc[:], in1=bc_s[:],
                                            op=mybir.AluOpType.add)
                    nrows = min(P, SHARD - b * P)
                    nc.sync.dma_start(out=out[b * P:b * P + nrows, :],
                                      in_=ob[:nrows, :])

                aggregation(g2_t, idx2_s, b2_s, l2_out, "B")

    mybir.codegen_inst_isa_subclasses(nc)
    _split_excess_waits(nc)
    return nc


def _prepare(x, edge_index, W1, b1, W2, b2, Wc, bc):
    import ml_dtypes
    bf = ml_dtypes.bfloat16
    x = np.asarray(x, np.float32)
    src = np.asarray(edge_index[0], dtype=np.int64)
    dst = np.asarray(edge_index[1], dtype=np.int64)
    deg = np.bincount(dst, minlength=N).astype(np.float32) + 1.0
    dinv = 1.0 / np.sqrt(deg)
    allsrc = np.concatenate([src, np.arange(N, dtype=np.int64)])
    alldst = np.concatenate([dst, np.arange(N, dtype=np.int64)])

    per_core = []
    tb = 0
    for cc in range(N_CORES):
        lo, hi = cc * SHARD, (cc + 1) * SHARD
        m = (alldst >= lo) & (alldst < hi)
        s = allsrc[m]
        d = alldst[m] - lo
        blk = d // P
        order = np.argsort(blk, kind="stable")
        s, d, blk = s[order], d[order], blk[order]
        counts = np.bincount(blk, minlength=NB)
        tb = max(tb, int(np.ceil(counts.max() / P)))
        per_core.append((s, d, counts))
    NT = NB * tb

    dinv_pad = np.ones(NPAD, np.float32)
    dinv_pad[:N] = dinv
    xt = np.zeros((IN_F, NPAD), np.float32)
    xt[:, :N] = x.T
    common = {
        "xt_a": xt[:P].astype(bf),
        "xt_b": xt[P:].astype(bf),
        "w1a": np.asarray(W1, np.float32)[:P].astype(bf),
        "w1b": np.asarray(W1, np.float32)[P:].astype(bf),
        "w2": np.asarray(W2, np.float32).astype(bf),
        "wc": np.asarray(Wc, np.float32).astype(bf),
        "b1rep": np.broadcast_to(np.asarray(b1, np.float32), (P, HID)).copy(),
        "b2rep": np.broadcast_to(np.asarray(b2, np.float32), (P, HID)).copy(),
        "bcrep": np.broadcast_to(np.asarray(bc, np.float32), (P, 2)).copy(),
        "iota_rep": np.broadcast_to(np.arange(P, dtype=np.float32),
                                    (P, P)).astype(bf),
        "ident": np.eye(P, dtype=np.float32).astype(bf),
        "dinv_nt": dinv_pad.reshape(NT1, P).T.copy(),
    }
    d2 = np.ones((N_CORES, SHARD_PAD), np.float32)
    for cc in range(N_CORES):
        d2[cc, :SHARD] = dinv[cc * SHARD:(cc + 1) * SHARD]
    common["dinv_l2"] = d2.reshape(N_CORES * NB, P).T.copy()

    in_maps = []
    for cc in range(N_CORES):
        s, d, counts = per_core[cc]
        idx_u = np.zeros((NT, P), np.int64)
        dst_u = np.full((NT, P), -1.0, np.float32)
        offs = np.concatenate([[0], np.cumsum(counts)])
        for b in range(NB):
            es = s[offs[b]:offs[b + 1]]
            ed = d[offs[b]:offs[b + 1]] % P
            nloc = len(es)
            buf_s = np.zeros(tb * P, np.int64)
            buf_d = np.full(tb * P, -1.0, np.float32)
            buf_s[:nloc] = es
            buf_d[:nloc] = ed
            idx_u[b * tb:(b + 1) * tb] = buf_s.reshape(tb, P)
            dst_u[b * tb:(b + 1) * tb] = buf_d.reshape(tb, P)
        idx_u = idx_u.T.copy()
        dst_u = dst_u.T.copy()
        idx2_u = (idx_u // SHARD) * SHARD_PAD + (idx_u % SHARD)
        lo = cc * SHARD
        dv = np.ones(SHARD_PAD, np.float32)
        dv[:SHARD] = dinv[lo:lo + SHARD]
        m = dict(common)
        m["idx1"] = idx_u.astype(np.int32)
        m["idx2"] = idx2_u.astype(np.int32)
        m["dstloc"] = dst_u.astype(bf)
        m["dinv_blk"] = dv.reshape(NB, P).T.copy()
        in_maps.append(m)
    return tb, in_maps


class _Runner:
    """Compile the Bass SPMD program once and execute it on cores 0-7 via
    the PJRT path (modeled on concourse.bass2jax.run_bass_via_pjrt)."""

    def __init__(self, nc, n_cores=8):
        import jax
        import concourse.mybir as mybir
        from jax.sharding import Mesh, PartitionSpec
        from jax.experimental.shard_map import shard_map
        from concourse.bass2jax import (_bass_exec_p, partition_id_tensor,
                                        install_neuronx_cc_hook)
        install_neuronx_cc_hook()
        self.jax = jax
        self.n_cores = n_cores
        in_names, out_names, out_avals = [], [], []
        partition_name = (nc.partition_id_tensor.name
                          if nc.partition_id_tensor else None)
        for alloc in nc.m.functions[0].allocations:
            if not isinstance(alloc, mybir.MemoryLocationSet):
                continue
            name = alloc.memorylocations[0].name
            if alloc.kind == "ExternalInput":
                if name != partition_name:
                    in_names.append(name)
            elif alloc.kind == "ExternalOutput":
                out_names.append(name)
                out_avals.append(jax.core.ShapedArray(
                    tuple(alloc.tensor_shape), mybir.dt.np(alloc.dtype)))
        self.in_names, self.out_names, self.out_avals = \
            in_names, out_names, out_avals
        n_params = len(in_names)
        all_in = list(in_names) + list(out_names)
        if partition_name is not None:
            all_in.append(partition_name)

        def _body(*args):
            operands = list(args)
            if partition_name is not None:
                operands.append(partition_id_tensor())
            outs = _bass_exec_p.bind(
                *operands, out_avals=tuple(out_avals), in_names=tuple(all_in),
                out_names=tuple(out_names), lowering_input_output_aliases=(),
                sim_require_finite=True, sim_require_nnan=True, nc=nc)
            return tuple(outs)

        devices = jax.devices()[:n_cores]
        self.mesh = Mesh(np.asarray(devices), ("core",))
        n_outs = len(out_avals)
        in_specs = (PartitionSpec("core"),) * (n_params + n_outs)
        out_specs = (PartitionSpec("core"),) * n_outs
        self.fn = jax.jit(
            shard_map(_body, mesh=self.mesh, in_specs=in_specs,
                      out_specs=out_specs, check_rep=False),
            keep_unused=True)

    def prep_inputs(self, in_maps):
        import jax
        from jax.sharding import NamedSharding, PartitionSpec
        concat = [np.concatenate([np.asarray(m[name]) for m in in_maps], axis=0)
                  for name in self.in_names]
        zeros = [np.zeros((self.n_cores * a.shape[0], *a.shape[1:]), a.dtype)
                 for a in self.out_avals]
        sharding = NamedSharding(self.mesh, PartitionSpec("core"))
        return [jax.device_put(a, sharding) for a in concat + zeros]

    def run(self, dev_args):
        outs = self.fn(*dev_args)
        self.jax.block_until_ready(outs)
        return outs

    def results(self, outs):
        res = []
        for c in range(self.n_cores):
            d = {}
            for i, name in enumerate(self.out_names):
                d[name] = np.asarray(outs[i]).reshape(
                    self.n_cores, *self.out_avals[i].shape)[c]
            res.append(d)
        return res


_CACHED = {}


def kernel(x, edge_index, W1, b1, W2, b2, Wc, bc):
    tb, in_maps = _prepare(x, edge_index, W1, b1, W2, b2, Wc, bc)
    key = tb
    if key not in _CACHED:
        nc = _build(tb)
        _CACHED[key] = _Runner(nc)
    r = _CACHED[key]
    dev = r.prep_inputs(in_maps)
    outs = r.run(dev)
    res = r.results(outs)
    full = np.concatenate([res[c]["out"] for c in range(N_CORES)], axis=0)
    return full.astype(np.float32)
